# revision 13
# baseline (speedup 1.0000x reference)
"""Trainium2 Bass kernel for nn_MultiHeadedAttention_19713899889501.

Strategy: pure data-parallel over batch (B=8 -> 8 NeuronCores), no
collectives.  Per core, one batch element:

  qagg[t] = sum_{j<5} q[t+j]                  (unweighted window sum)
  kagg[t] = sum_j softmax_j(<k[t+4],k[t+j]>/sqrt(D)) k[t+j]
  vlin    = v[4:] @ W0 + b0
  out     = MHA(qagg, kagg, vlin) @ Wout + bout

Device layout is fully transposed (d on partitions, seq on free dim):
  - qaggT via DVE shift-add tree on host-pre-transposed qT
  - local scores via DVE products + PE ones-reduce (shifted-basis lhsT)
  - softmax normalization of the local weights is folded into a kagg
    pre-scale (1/wsum); 1/sqrt(DK) goes into the ACT exp scale immediate
  - QK^T head-paired: two K=64 matmuls on disjoint PE row groups run
    concurrently (rows 0-63 = even head, 64-127 = odd head)
  - exp on ScalarE over (128, 1024) two-bank PSUM reads, bf16 out
  - att@V with lhsT = [v_head 64 cols | ones 64 cols] so the softmax
    denominators come out replicated across 64 partitions for free
  - output linear with bias via K=1 ones-row matmul

The t/s grid is padded 2044 -> 2048; padded key positions are nulled by
zeroing their vlin rows (including the ones column); padded t rows are
simply not written back.
"""
import sys

if "/opt/trn_rl_repo" not in sys.path:
    sys.path.insert(0, "/opt/trn_rl_repo")

import numpy as np
import ml_dtypes

import concourse.bass as bass
import concourse.tile as tile
import concourse.mybir as mybir
from concourse import bacc
from concourse.bass_utils import run_bass_kernel_spmd

BF16 = ml_dtypes.bfloat16
F32 = mybir.dt.float32
BF = mybir.dt.bfloat16
AF = mybir.ActivationFunctionType

B, S, D, H, L = 8, 2048, 512, 8, 5
DK = D // H           # 64
SP = S - L + 1        # 2044 true output positions
SPP = 2048            # padded t/s grid
SQ = S + 8            # padded qT/kT width (2056)
NCH = 4               # d chunks of 128
NT = 4                # t chunks of 512
NS = 16               # s tiles of 128
N_CORES = 8

_PROGRAM = None


def _build_core_program():
    nc = bacc.Bacc("TRN2", target_bir_lowering=False, debug=False)

    qT = nc.dram_tensor("qT", [D, SQ], BF, kind="ExternalInput").ap()
    kT = nc.dram_tensor("kT", [D, SQ], BF, kind="ExternalInput").ap()
    vT = nc.dram_tensor("vT", [D, SPP], BF, kind="ExternalInput").ap()
    W0 = nc.dram_tensor("W0", [D, D], BF, kind="ExternalInput").ap()
    Wout = nc.dram_tensor("Wout", [D, D], BF, kind="ExternalInput").ap()
    b0 = nc.dram_tensor("b0", [1, D], BF, kind="ExternalInput").ap()
    bout = nc.dram_tensor("bout", [1, D], BF, kind="ExternalInput").ap()
    zpad = nc.dram_tensor("zpad", [4, 1024], BF, kind="ExternalInput").ap()
    out = nc.dram_tensor("out", [SP, D], F32, kind="ExternalOutput").ap()

    with tile.TileContext(nc) as tc:
        _build(tc, qT, kT, vT, W0, Wout, b0, bout, zpad, out)
    nc.compile()
    return nc


def _build(tc, qT, kT, vT, W0, Wout, b0, bout, zpad, out):
    nc = tc.nc
    from contextlib import ExitStack

    inv_sqrt_d = float(1.0 / np.sqrt(np.float32(D)))
    inv_sqrt_dk = float(1.0 / np.sqrt(np.float32(DK)))

    with ExitStack() as ctx:
        pers = ctx.enter_context(tc.tile_pool(name="pers", bufs=1))

        # ---------------- constants / weights ----------------
        ones_row = pers.tile([1, 128], BF, tag="ones_row")
        nc.vector.memset(ones_row[:], 1.0)
        ones_col = pers.tile([128, 1], BF, tag="ones_col")
        nc.vector.memset(ones_col[:], 1.0)

        b0_sb = pers.tile([1, D], BF, tag="b0")
        nc.sync.dma_start(b0_sb[:], b0[:])
        bout_sb = pers.tile([1, D], BF, tag="bout")
        nc.sync.dma_start(bout_sb[:], bout[:])

        kTb = []
        for c in range(NCH):
            t = pers.tile([128, SQ], BF, tag=f"kTb{c}")
            nc.sync.dma_start(t[:], kT[c * 128:(c + 1) * 128, :])
            kTb.append(t)
        W0b = []
        Woutb = []
        for c in range(NCH):
            t = pers.tile([128, D], BF, tag=f"W0b{c}")
            nc.sync.dma_start(t[:], W0[c * 128:(c + 1) * 128, :])
            W0b.append(t)
            t = pers.tile([128, D], BF, tag=f"Woutb{c}")
            nc.sync.dma_start(t[:], Wout[c * 128:(c + 1) * 128, :])
            Woutb.append(t)

        # persistent results of the prefix
        qaggT = []
        kaggT = []
        vlin_sb = []
        XhatT = []
        for c in range(NCH):
            t = pers.tile([128, SPP], BF, tag=f"qaggT{c}")
            qaggT.append(t)
            t = pers.tile([128, SPP], BF, tag=f"kaggT{c}")
            kaggT.append(t)
            t = pers.tile([128, SPP], BF, tag=f"XhatT{c}")
            XhatT.append(t)
        # per head h: cols [128h:128h+64] = v head cols, [128h+64:128h+128] = ones
        for st in range(NS):
            t = pers.tile([128, 1024], BF, tag=f"vlin{st}")
            vlin_sb.append(t)

        # ================ prefix phase A: vlin ============
        with ExitStack() as pre:
            vtp = pre.enter_context(tc.tile_pool(name="vtp", bufs=1))
            pre_ps = pre.enter_context(
                tc.tile_pool(name="pre_ps", bufs=2, space="PSUM"))

            vtbs = []
            for c in range(NCH):
                vtb = vtp.tile([128, SPP], BF, tag=f"vTb{c}")
                nc.sync.dma_start(vtb[:], vT[c * 128:(c + 1) * 128, :])
                vtbs.append(vtb)
            for st in range(NS):
                ps = pre_ps.tile([128, 512], F32, tag="vlin_ps")
                for c in range(NCH):
                    nc.tensor.matmul(
                        ps[:], vtbs[c][:, st * 128:(st + 1) * 128], W0b[c][:],
                        start=(c == 0), stop=False,
                    )
                nc.tensor.matmul(ps[:], ones_row[:], b0_sb[:],
                                 start=False, stop=True)
                vre = vlin_sb[st].rearrange("p (h u) -> p h u", u=128)
                psr = ps.rearrange("p (h u) -> p h u", u=64)
                nc.scalar.activation(vre[:, :, 0:64], psr[:], AF.Copy)
                nc.vector.memset(vre[:, :, 64:128], 1.0)
                if st == NS - 1:
                    # zero padded key rows (engine ops can't address base 124;
                    # DMA is address-based and can)
                    nc.sync.dma_start(vlin_sb[st][124:128, :], zpad[:])

        # ================ prefix phase B: qagg tree ============
        with ExitStack() as pre:
            treep = pre.enter_context(tc.tile_pool(name="treep", bufs=2))
            for c in range(NCH):
                x = treep.tile([128, SQ], BF, tag="qT_in")
                nc.sync.dma_start(x[:], qT[c * 128:(c + 1) * 128, :])
                s1 = treep.tile([128, 2052], BF, tag="tree1")
                nc.vector.tensor_add(s1[:], x[:, 0:2052], x[:, 1:2053])
                s2 = treep.tile([128, SPP], BF, tag="tree2")
                nc.vector.tensor_add(s2[:], s1[:, 0:SPP], s1[:, 2:2 + SPP])
                nc.vector.tensor_add(qaggT[c][:], s2[:], x[:, 4:4 + SPP])

        # ================ prefix phase C: local scores + kagg ============
        with ExitStack() as pre:
            prodp = pre.enter_context(tc.tile_pool(name="prodp", bufs=3))
            rowp = pre.enter_context(tc.tile_pool(name="rowp", bufs=2))
            ebp = pre.enter_context(tc.tile_pool(name="ebp", bufs=2))
            kwp = pre.enter_context(tc.tile_pool(name="kwp", bufs=3))
            scr_ps = pre.enter_context(
                tc.tile_pool(name="scr_ps", bufs=1, space="PSUM"))

            # ---- local score dots + per-t-quarter kagg chain ----
            # scr4 holds the 5 lags side by side in the free dim, all on
            # partition 0: scr4[0, 512*j + t] = sum_d kT[d,t+4]*kT[d,t+j]
            for t4 in range(NT):
                sl = slice(t4 * 512, (t4 + 1) * 512)
                scr4 = scr_ps.tile([1, 5 * 512], F32, tag="scr4")
                for j in range(L):
                    for c in range(NCH):
                        p = prodp.tile([128, 512], BF, tag="prod")
                        nc.vector.tensor_mul(
                            p[:],
                            kTb[c][:, t4 * 512 + 4:t4 * 512 + 4 + 512],
                            kTb[c][:, t4 * 512 + j:t4 * 512 + j + 512])
                        nc.tensor.matmul(
                            scr4[:, j * 512:(j + 1) * 512],
                            ones_col[:], p[:],
                            start=(c == 0), stop=(c == NCH - 1),
                        )

                # e4[0, 512*j + t] = exp(scr4 / sqrt(D)) for all 5 lags at once
                e4 = rowp.tile([1, 5 * 512], F32, tag="e4")
                nc.scalar.activation(e4[:], scr4[:], AF.Exp, scale=inv_sqrt_d)
                w1 = rowp.tile([1, 512], F32, tag="w1")
                nc.vector.tensor_add(w1[:], e4[:, 0:512], e4[:, 512:1024])
                w2 = rowp.tile([1, 512], F32, tag="w2")
                nc.vector.tensor_add(w2[:], e4[:, 1024:1536], e4[:, 1536:2048])
                w3 = rowp.tile([1, 512], F32, tag="w3")
                nc.vector.tensor_add(w3[:], w1[:], w2[:])
                wsum = rowp.tile([1, 512], F32, tag="wsum")
                nc.vector.tensor_add(wsum[:], w3[:], e4[:, 2048:2560])
                rrow = rowp.tile([1, 512], F32, tag="rrow")
                nc.vector.reciprocal(rrow[:], wsum[:])

                ebs = []
                for j in range(L):
                    eb = ebp.tile([128, 512], F32, tag=f"eb{j}")
                    nc.gpsimd.partition_broadcast(eb[:], e4[:, j * 512:(j + 1) * 512])
                    ebs.append(eb)
                recipb = ebp.tile([128, 512], F32, tag="recipb")
                nc.gpsimd.partition_broadcast(recipb[:], rrow[:])

                for c in range(NCH):
                    acc = kwp.tile([128, 512], F32, tag="kacc")
                    nc.vector.tensor_mul(
                        acc[:], kTb[c][:, t4 * 512:t4 * 512 + 512], ebs[0][:])
                    for j in range(1, L):
                        term = kwp.tile([128, 512], F32, tag="kterm")
                        nc.vector.tensor_mul(
                            term[:], kTb[c][:, t4 * 512 + j:t4 * 512 + j + 512],
                            ebs[j][:])
                        acc2 = kwp.tile([128, 512], F32, tag="kacc")
                        nc.vector.tensor_add(acc2[:], acc[:], term[:])
                        acc = acc2
                    nc.vector.tensor_mul(kaggT[c][:, sl], acc[:], recipb[:])

        # ======================== SDPA main loop ============================
        with ExitStack() as main:
            pap = main.enter_context(tc.tile_pool(name="pap", bufs=3))
            rxp = main.enter_context(tc.tile_pool(name="rxp", bufs=2))
            osbp = main.enter_context(tc.tile_pool(name="osbp", bufs=2))
            qk_ps = main.enter_context(
                tc.tile_pool(name="qk_ps", bufs=2, space="PSUM"))
            x_ps_pool = main.enter_context(
                tc.tile_pool(name="x_ps", bufs=2, space="PSUM"))

            for pair in range(H // 2):
                c = pair                  # chunk c holds heads 2c (rows 0:64) and 2c+1 (64:128)
                hA, hB = 2 * pair, 2 * pair + 1
                for tcx in range(NT):
                    tsl = slice(tcx * 512, (tcx + 1) * 512)
                    xA = x_ps_pool.tile([128, 512], F32, tag="xA")
                    xB = x_ps_pool.tile([128, 512], F32, tag="xB")
                    for st in range(NS):
                        ssl = slice(st * 128, (st + 1) * 128)
                        p_ps = qk_ps.tile([128, 1024], F32, tag="p_ps")
                        nc.tensor.matmul(
                            p_ps[:, 0:512],
                            kaggT[c][0:64, ssl], qaggT[c][0:64, tsl],
                            start=True, stop=True,
                        )
                        nc.tensor.matmul(
                            p_ps[:, 512:1024],
                            kaggT[c][64:128, ssl], qaggT[c][64:128, tsl],
                            start=True, stop=True,
                        )
                        pa = pap.tile([128, 1024], BF, tag="pa")
                        nc.scalar.activation(pa[:], p_ps[:], AF.Exp,
                                             scale=inv_sqrt_dk)
                        nc.tensor.matmul(
                            xA[:], vlin_sb[st][:, hA * 128:(hA + 1) * 128],
                            pa[:, 0:512],
                            start=(st == 0), stop=(st == NS - 1),
                        )
                        nc.tensor.matmul(
                            xB[:], vlin_sb[st][:, hB * 128:(hB + 1) * 128],
                            pa[:, 512:1024],
                            start=(st == 0), stop=(st == NS - 1),
                        )
                    # normalize; rows 0:64 = X^T_h, 64:128 = replicated denoms
                    rxA = rxp.tile([64, 512], F32, tag="rxA")
                    nc.vector.reciprocal(rxA[:], xA[64:128, :])
                    nc.vector.tensor_mul(XhatT[c][0:64, tsl], xA[0:64, :], rxA[:])
                    rxB = rxp.tile([64, 512], F32, tag="rxB")
                    nc.vector.reciprocal(rxB[:], xB[64:128, :])
                    nc.vector.tensor_mul(XhatT[c][64:128, tsl], xB[0:64, :], rxB[:])

            # ---------------- output linear ----------------
            for tb in range(NS):
                o_ps = x_ps_pool.tile([128, 512], F32, tag="xA")
                for c in range(NCH):
                    nc.tensor.matmul(
                        o_ps[:], XhatT[c][:, tb * 128:(tb + 1) * 128], Woutb[c][:],
                        start=(c == 0), stop=False,
                    )
                nc.tensor.matmul(o_ps[:], ones_row[:], bout_sb[:],
                                 start=False, stop=True)
                o_sb = osbp.tile([128, 512], F32, tag="o_sb")
                nc.scalar.activation(o_sb[:], o_ps[:], AF.Copy)
                rows = 128 if tb < NS - 1 else SP - 128 * (NS - 1)
                nc.sync.dma_start(out[tb * 128: tb * 128 + rows, :],
                                  o_sb[0:rows, :])


def _get_program():
    global _PROGRAM
    if _PROGRAM is None:
        _PROGRAM = _build_core_program()
    return _PROGRAM


def _prep_core_inputs(q, k, v, W0, b0, Wout, bout):
    """Host-side layout prep for one batch element (layout/dtype only)."""
    qTp = np.zeros((D, SQ), BF16)
    qTp[:, 0:S] = np.ascontiguousarray(q.T).astype(BF16)
    kTp = np.zeros((D, SQ), BF16)
    kTp[:, 0:S] = np.ascontiguousarray(k.T).astype(BF16)
    vTp = np.zeros((D, SPP), BF16)
    vTp[:, 0:S - 4] = np.ascontiguousarray(v[4:].T).astype(BF16)
    return {
        "qT": qTp,
        "kT": kTp,
        "vT": vTp,
        "W0": W0.astype(BF16),
        "Wout": Wout.astype(BF16),
        "b0": b0.reshape(1, D).astype(BF16),
        "bout": bout.reshape(1, D).astype(BF16),
        "zpad": np.zeros((4, 1024), BF16),
    }


def kernel(query, key, value, W0, b0, Wout, bout):
    query = np.asarray(query, np.float32)
    key = np.asarray(key, np.float32)
    value = np.asarray(value, np.float32)
    W0 = np.asarray(W0, np.float32)
    b0 = np.asarray(b0, np.float32)
    Wout = np.asarray(Wout, np.float32)
    bout = np.asarray(bout, np.float32)

    nc = _get_program()
    in_maps = [
        _prep_core_inputs(query[b], key[b], value[b], W0, b0, Wout, bout)
        for b in range(B)
    ]
    res = run_bass_kernel_spmd(nc, in_maps, list(range(N_CORES)))
    return np.stack([res.results[b]["out"] for b in range(B)], axis=0)
